# revision 7
# baseline (speedup 1.0000x reference)
"""MixtureSageLayer Trainium2 kernel: scatter-mean GNN aggregation + 8-expert
dense layer + residual, sharded over 8 NeuronCores by destination node.

kernel(x, edge_index, weights) -> [100000, 8, 64] float32

Per-core plan (SPMD, one program, per-core data):
  - dest nodes sharded: core c owns rows [c*12500, (c+1)*12500)
  - edges routed to their dest core, grouped by (batch of 8 dest-windows,
    source-chunk of 25000, dest-window of 128), each (window, chunk) run
    padded to a multiple of 128 edges (pad slots gather row 0 of the chunk
    and are masked out by d=-1 / invdeg=0)
  - bulk dma_gather (int16 idxs, SWDGE) fetches messages from a bf16
    row-duplicated copy of x (elem = 256B) into SBUF
  - per 128-edge tile: one fused DVE op builds Q[e, i] = (d[e]==i)*invdeg[e]
    (bf16), one TensorE bf16 matmul accumulates aggT[64, 4 windows * 128]
    in PSUM (scatter-mean, collision free)
  - per window: two float32r matmuls compute
      out[128, 512] = aggT.T @ Wa_r + xT_w.T @ (Wx_r + I)
    (the +I folds in the residual), PSUM -> SBUF -> DMA to the output shard
"""
import numpy as np
import ml_dtypes

import concourse.bacc as bacc
import concourse.mybir as mybir
import concourse.tile as tile
from concourse.bass_utils import run_bass_kernel_spmd

N_NODES = 100000
N_EDGES = 1000000
D = 64
K = 8
N_CORES = 8
NPC = N_NODES // N_CORES          # 12500 dest nodes per core
P = 128
NW = (NPC + P - 1) // P           # 98 windows per core
NPC_PAD = NW * P                  # 12544
WB = 8                            # windows per gather batch
NBATCH = (NW + WB - 1) // WB      # 13
NCH = 4                           # source chunks (int16 idx limit)
CH = 25000                        # chunk size
GRP = 4                           # windows per PSUM-A tile ([64, 512])

f32 = mybir.dt.float32
f32r = mybir.dt.float32r
bf16 = mybir.dt.bfloat16
i16 = mybir.dt.int16

MAX_WAITS = 1

_CACHE = {}


def _split_sync_waits(nc, max_waits=MAX_WAITS):
    """The walrus codegen in this toolchain accepts very few sync-wait
    commands per instruction; spread extras onto preceding NoOps."""
    for f in nc.m.functions:
        for b in f.blocks:
            new_insts = []
            for inst in b.instructions:
                si = inst.sync_info
                if si is not None and si.on_wait and len(si.on_wait) > max_waits:
                    waits = list(si.on_wait)
                    chunks = [waits[i:i + max_waits]
                              for i in range(0, len(waits), max_waits)]
                    for k, chunk in enumerate(chunks[:-1]):
                        new_insts.append(mybir.InstNoOp(
                            name=f"{inst.name}-sw{k}",
                            sync_info=mybir.SyncInfo(on_wait=chunk, on_update=[]),
                            bass_nofuse=True,
                            engine=inst.engine,
                        ))
                    si.on_wait = chunks[-1]
                new_insts.append(inst)
            b.instructions = new_insts


def _wrap_idxs(seg):
    """dma_gather index layout: stream pos k -> [k % 16, k // 16], x8 tiled."""
    a = seg.reshape(-1, 16).T.astype(np.int16)
    return np.tile(a, (8, 1))


def _build_plan(edge_index):
    """Host-side routing: per-core padded edge streams + shared static shape."""
    row = np.asarray(edge_index[0], dtype=np.int64)
    col = np.asarray(edge_index[1], dtype=np.int64)

    deg = np.bincount(row, minlength=N_NODES).astype(np.float32)
    inv_global = (1.0 / np.maximum(deg, 1.0)).astype(np.float32)

    core_of = row // NPC
    per_core = []
    cnts = np.zeros((N_CORES, NW * NCH), np.int64)
    for c in range(N_CORES):
        sel = core_of == c
        rl = (row[sel] - c * NPC).astype(np.int64)
        cl = col[sel]
        w = rl >> 7
        j = cl // CH
        key = (w * NCH + j).astype(np.int64)
        cnts[c] = np.bincount(key, minlength=NW * NCH)
        per_core.append((rl, cl, key))

    # shared static tile counts per (window, chunk)
    t_wj = np.maximum(1, -(-cnts.max(axis=0) // P)).reshape(NW, NCH)

    # stream order: for batch b: for chunk j: for window w in batch
    pair_order = []
    for b in range(NBATCH):
        ws = range(b * WB, min((b + 1) * WB, NW))
        for j in range(NCH):
            for w in ws:
                pair_order.append((w, j))
    tile_off = np.zeros((NW, NCH), np.int64)
    acc = 0
    for (w, j) in pair_order:
        tile_off[w, j] = acc
        acc += t_wj[w, j]
    T_total = int(acc)

    # per-(batch, chunk) gather segments (tile ranges)
    segs = []     # (batch, chunk, tile_start, tile_end)
    for b in range(NBATCH):
        ws = list(range(b * WB, min((b + 1) * WB, NW)))
        for j in range(NCH):
            s0 = tile_off[ws[0], j]
            s1 = tile_off[ws[-1], j] + t_wj[ws[-1], j]
            segs.append((b, j, int(s0), int(s1)))

    # per-core streams
    cores = []
    for c in range(N_CORES):
        rl, cl, key = per_core[c]
        order = np.argsort(key, kind="stable")
        key_s = key[order]
        rl_s = rl[order]
        cl_s = cl[order]
        grp_start = np.zeros(NW * NCH, np.int64)
        np.cumsum(cnts[c], out=grp_start[0:])
        grp_start = np.concatenate([[0], grp_start[:-1]])
        rank = np.arange(len(key_s)) - grp_start[key_s]
        w_s = key_s // NCH
        j_s = key_s % NCH
        pos = tile_off[w_s, j_s] * P + rank

        stream_idx = np.zeros(T_total * P, np.int16)
        stream_d = np.full(T_total * P, -1.0, np.float32)
        stream_inv = np.zeros(T_total * P, np.float32)
        stream_idx[pos] = (cl_s - j_s * CH).astype(np.int16)
        stream_d[pos] = (rl_s - w_s * P).astype(np.float32)
        stream_inv[pos] = inv_global[rl_s + c * NPC]

        idx_wrapped = np.concatenate(
            [_wrap_idxs(stream_idx[s0 * P:s1 * P]) for (_, _, s0, s1) in segs],
            axis=1)
        d_tiled = stream_d.reshape(T_total, P).T.copy()
        inv_tiled = stream_inv.reshape(T_total, P).T.copy()
        cores.append((idx_wrapped, d_tiled, inv_tiled))

    return t_wj, tile_off, T_total, segs, cores


def _build_program(t_wj, tile_off, T_total, segs, split_waits=True,
                   do_gather=True, do_compute=True):
    nc = bacc.Bacc("TRN2")

    ST16 = sum((s1 - s0) * P // 16 for (_, _, s0, s1) in segs)
    xdup_d = nc.dram_tensor("xdup", [N_NODES, 2 * D], bf16, kind="ExternalInput")
    xt_d = nc.dram_tensor("xt", [D, NPC_PAD], f32r, kind="ExternalInput")
    idx_d = nc.dram_tensor("idx", [P, ST16], i16, kind="ExternalInput")
    d_d = nc.dram_tensor("d", [P, T_total], f32, kind="ExternalInput")
    inv_d = nc.dram_tensor("inv", [P, T_total], f32, kind="ExternalInput")
    iota_d = nc.dram_tensor("iota", [P, P], bf16, kind="ExternalInput")
    wa_d = nc.dram_tensor("wa", [D, K * D], f32r, kind="ExternalInput")
    wxi_d = nc.dram_tensor("wxi", [D, K * D], f32r, kind="ExternalInput")
    out_d = nc.dram_tensor("out", [NPC_PAD, K * D], f32, kind="ExternalOutput")

    # gather segment offsets in the idx tensor (int16 columns)
    seg_idx_off = []
    o = 0
    for (_, _, s0, s1) in segs:
        seg_idx_off.append(o)
        o += (s1 - s0) * P // 16

    # batch tile ranges
    batch_range = []
    for b in range(NBATCH):
        bsegs = [s for s in segs if s[0] == b]
        batch_range.append((bsegs[0][2], bsegs[-1][3]))
    S_max = max(s1 - s0 for (s0, s1) in batch_range)

    with tile.TileContext(nc) as tc:
        with (
            tc.tile_pool(name="const", bufs=1) as cpool,
            tc.tile_pool(name="msg", bufs=2) as mpool,
            tc.tile_pool(name="q", bufs=6) as qpool,
            tc.tile_pool(name="agg", bufs=4) as apool,
            tc.tile_pool(name="outs", bufs=4) as opool,
            tc.tile_pool(name="psA", bufs=3, space="PSUM") as pApool,
            tc.tile_pool(name="psB", bufs=3, space="PSUM") as pBpool,
        ):
            idx_t = cpool.tile([P, ST16], i16)
            d_t = cpool.tile([P, T_total], f32)
            inv_t = cpool.tile([P, T_total], f32)
            iota_t = cpool.tile([P, P], bf16)
            xt_t = cpool.tile([D, NPC_PAD], f32r)
            wa_t = cpool.tile([D, K * D], f32r)
            wxi_t = cpool.tile([D, K * D], f32r)
            nc.sync.dma_start(out=idx_t[:], in_=idx_d[:])
            nc.sync.dma_start(out=d_t[:], in_=d_d[:])
            nc.sync.dma_start(out=inv_t[:], in_=inv_d[:])
            nc.sync.dma_start(out=iota_t[:], in_=iota_d[:])
            nc.sync.dma_start(out=xt_t[:], in_=xt_d[:])
            nc.sync.dma_start(out=wa_t[:], in_=wa_d[:])
            nc.sync.dma_start(out=wxi_t[:], in_=wxi_d[:])

            for b in range(NBATCH):
                bs0, bs1 = batch_range[b]
                msg_t = mpool.tile([P, S_max, 2 * D], bf16, tag="msg")

                for si, (bb, j, s0, s1) in enumerate(segs):
                    if bb != b:
                        continue
                    if not do_gather:
                        continue
                    L = (s1 - s0) * P
                    io = seg_idx_off[si]
                    nc.gpsimd.dma_gather(
                        msg_t[:, s0 - bs0:s1 - bs0, :],
                        xdup_d[CH * j:, :] if j else xdup_d[:],
                        idx_t[:, io:io + L // 16],
                        L, L, 2 * D,
                        single_packet=False,
                    )

                if not do_gather:
                    nc.vector.memset(msg_t[:, 0, :], 0.0)
                ws = list(range(b * WB, min((b + 1) * WB, NW)))
                if not do_compute:
                    continue
                # window-major consumption: each window's PSUM group is
                # opened and closed consecutively, then fed to stage B.
                for w in ws:
                    psA = pApool.tile([D, P], mybir.dt.float32,
                                      space="PSUM", tag="psA")
                    for j in range(NCH):
                        for t in range(int(t_wj[w, j])):
                            tt = int(tile_off[w, j]) + t
                            q_t = qpool.tile([P, P], bf16, tag="q")
                            nc.vector.tensor_scalar(
                                out=q_t[:],
                                in0=iota_t[:],
                                scalar1=d_t[:, tt:tt + 1],
                                scalar2=inv_t[:, tt:tt + 1],
                                op0=mybir.AluOpType.is_equal,
                                op1=mybir.AluOpType.mult,
                            )
                            nc.tensor.matmul(
                                out=psA[:],
                                lhsT=msg_t[:, tt - bs0, 0:D],
                                rhs=q_t[:],
                                start=(j == 0 and t == 0),
                                stop=(j == NCH - 1 and t == int(t_wj[w, j]) - 1),
                            )

                    aggT_t = apool.tile([D, P], f32r, tag="agg")
                    nc.scalar.mul(aggT_t[:], psA[:], 1.0)
                    psB = pBpool.tile([P, K * D], mybir.dt.float32,
                                      space="PSUM", tag="psB")
                    nc.tensor.matmul(out=psB[:], lhsT=aggT_t[:], rhs=wa_t[:],
                                     start=True, stop=False)
                    nc.tensor.matmul(out=psB[:],
                                     lhsT=xt_t[:, w * P:(w + 1) * P],
                                     rhs=wxi_t[:], start=False, stop=True)
                    out_t = opool.tile([P, K * D], f32, tag="out")
                    nc.scalar.mul(out_t[:], psB[:], 1.0)
                    nc.sync.dma_start(out=out_d[w * P:(w + 1) * P, :],
                                      in_=out_t[:])

    nc.compile()
    if split_waits:
        _split_sync_waits(nc)
    return nc


def kernel(x, edge_index, weights):
    x = np.asarray(x, dtype=np.float32)
    weights = np.asarray(weights, dtype=np.float32)

    t_wj, tile_off, T_total, segs, cores = _build_plan(edge_index)

    shape_key = (T_total, tuple(t_wj.ravel().tolist()))
    if shape_key in _CACHE:
        nc = _CACHE[shape_key]
    else:
        nc = _build_program(t_wj, tile_off, T_total, segs)
        _CACHE.clear()
        _CACHE[shape_key] = nc

    x_bf = x.astype(ml_dtypes.bfloat16)
    xdup = np.concatenate([x_bf, x_bf], axis=1)          # [N, 128] bf16

    iota = np.broadcast_to(
        np.arange(P, dtype=np.float32), (P, P)).astype(ml_dtypes.bfloat16)

    wa = np.ascontiguousarray(
        weights[:, :D, :].transpose(1, 0, 2).reshape(D, K * D))
    wx = weights[:, D:, :].transpose(1, 0, 2).reshape(D, K * D).copy()
    eye = np.eye(D, dtype=np.float32)
    for k in range(K):
        wx[:, k * D:(k + 1) * D] += eye

    in_maps = []
    for c in range(N_CORES):
        idx_wrapped, d_tiled, inv_tiled = cores[c]
        xt = np.zeros((D, NPC_PAD), np.float32)
        xt[:, :NPC] = x[c * NPC:(c + 1) * NPC].T
        in_maps.append({
            "xdup": xdup,
            "xt": xt,
            "idx": idx_wrapped,
            "d": d_tiled,
            "inv": inv_tiled,
            "iota": np.ascontiguousarray(iota),
            "wa": wa,
            "wxi": wx,
        })

    res = run_bass_kernel_spmd(nc, in_maps, core_ids=list(range(N_CORES)))

    out = np.empty((N_NODES, K, D), np.float32)
    for c in range(N_CORES):
        oc = res.results[c]["out"][:NPC]                 # [12500, 512]
        out[c * NPC:(c + 1) * NPC] = oc.reshape(NPC, K, D)
    return out


# revision 13
# speedup vs baseline: 2.6753x; 2.6753x over previous
"""MixtureSageLayer Trainium2 kernel: scatter-mean GNN aggregation + 8-expert
dense layer + residual, sharded over 8 NeuronCores by destination node.

kernel(x, edge_index, weights) -> [100000, 8, 64] float32

Per-core plan (SPMD, one program, per-core data):
  - dest nodes sharded: core c owns rows [c*12500, (c+1)*12500)
  - edges routed to their dest core, grouped by (batch of 8 dest-windows,
    source-chunk of 25000, dest-window of 128), each (window, chunk) run
    padded to a multiple of 128 edges (pad slots gather row 0 of the chunk
    and are masked out by d=-1 / invdeg=0)
  - bulk dma_gather (int16 idxs, SWDGE) fetches messages from a bf16
    row-duplicated copy of x (elem = 256B) into SBUF
  - per 128-edge tile: one fused DVE op builds Q[e, i] = (d[e]==i)*invdeg[e]
    (bf16), one TensorE bf16 matmul accumulates aggT[64, 4 windows * 128]
    in PSUM (scatter-mean, collision free)
  - per window: two float32r matmuls compute
      out[128, 512] = aggT.T @ Wa_r + xT_w.T @ (Wx_r + I)
    (the +I folds in the residual), PSUM -> SBUF -> DMA to the output shard
"""
import numpy as np
import ml_dtypes

import concourse.bass as bass
import concourse.bacc as bacc
import concourse.mybir as mybir
import concourse.tile as tile
from concourse.bass_utils import run_bass_kernel_spmd

N_NODES = 100000
N_EDGES = 1000000
D = 64
K = 8
N_CORES = 8
NPC = N_NODES // N_CORES          # 12500 dest nodes per core
P = 128
NW = (NPC + P - 1) // P           # 98 windows per core
NPC_PAD = NW * P                  # 12544
WB = 8                            # windows per gather batch
NBATCH = (NW + WB - 1) // WB      # 13
NCH = 4                           # source chunks (int16 idx limit)
CH = 25000                        # chunk size
GRP = 4                           # windows per PSUM-A tile ([64, 512])

f32 = mybir.dt.float32
f32r = mybir.dt.float32r
bf16 = mybir.dt.bfloat16
i16 = mybir.dt.int16

MAX_WAITS = 1

_CACHE = {}


def _split_sync_waits(nc, max_waits=MAX_WAITS):
    """The walrus codegen in this toolchain accepts very few sync-wait
    commands per instruction; spread extras onto preceding NoOps."""
    for f in nc.m.functions:
        for b in f.blocks:
            new_insts = []
            for inst in b.instructions:
                si = inst.sync_info
                if si is not None and si.on_wait and len(si.on_wait) > max_waits:
                    waits = list(si.on_wait)
                    chunks = [waits[i:i + max_waits]
                              for i in range(0, len(waits), max_waits)]
                    for k, chunk in enumerate(chunks[:-1]):
                        new_insts.append(mybir.InstNoOp(
                            name=f"{inst.name}-sw{k}",
                            sync_info=mybir.SyncInfo(on_wait=chunk, on_update=[]),
                            bass_nofuse=True,
                            engine=inst.engine,
                        ))
                    si.on_wait = chunks[-1]
                new_insts.append(inst)
            b.instructions = new_insts


def _wrap_idxs(seg):
    """dma_gather index layout: stream pos k -> [k % 16, k // 16], x8 tiled."""
    a = seg.reshape(-1, 16).T.astype(np.int16)
    return np.tile(a, (8, 1))


def _build_plan(edge_index):
    """Host-side routing: per-core padded edge streams + shared static shape."""
    row = np.asarray(edge_index[0], dtype=np.int64)
    col = np.asarray(edge_index[1], dtype=np.int64)

    deg = np.bincount(row, minlength=N_NODES).astype(np.float32)
    inv_global = (1.0 / np.maximum(deg, 1.0)).astype(np.float32)

    core_of = row // NPC
    per_core = []
    cnts = np.zeros((N_CORES, NW * NCH), np.int64)
    for c in range(N_CORES):
        sel = core_of == c
        rl = (row[sel] - c * NPC).astype(np.int64)
        cl = col[sel]
        w = rl >> 7
        j = cl // CH
        key = (w * NCH + j).astype(np.int64)
        cnts[c] = np.bincount(key, minlength=NW * NCH)
        per_core.append((rl, cl, key))

    # shared static tile counts per (window, chunk)
    t_wj = np.maximum(1, -(-cnts.max(axis=0) // P)).reshape(NW, NCH)

    # stream order: for batch b: for chunk j: for window w in batch
    pair_order = []
    for b in range(NBATCH):
        ws = range(b * WB, min((b + 1) * WB, NW))
        for j in range(NCH):
            for w in ws:
                pair_order.append((w, j))
    tile_off = np.zeros((NW, NCH), np.int64)
    acc = 0
    for (w, j) in pair_order:
        tile_off[w, j] = acc
        acc += t_wj[w, j]
    T_total = int(acc)

    # per-(batch, chunk) gather segments (tile ranges)
    segs = []     # (batch, chunk, tile_start, tile_end)
    for b in range(NBATCH):
        ws = list(range(b * WB, min((b + 1) * WB, NW)))
        for j in range(NCH):
            s0 = tile_off[ws[0], j]
            s1 = tile_off[ws[-1], j] + t_wj[ws[-1], j]
            segs.append((b, j, int(s0), int(s1)))

    # per-core streams
    cores = []
    for c in range(N_CORES):
        rl, cl, key = per_core[c]
        order = np.argsort(key, kind="stable")
        key_s = key[order]
        rl_s = rl[order]
        cl_s = cl[order]
        grp_start = np.zeros(NW * NCH, np.int64)
        np.cumsum(cnts[c], out=grp_start[0:])
        grp_start = np.concatenate([[0], grp_start[:-1]])
        rank = np.arange(len(key_s)) - grp_start[key_s]
        w_s = key_s // NCH
        j_s = key_s % NCH
        pos = tile_off[w_s, j_s] * P + rank

        stream_idx = np.zeros(T_total * P, np.int16)
        stream_d = np.full(T_total * P, -1.0, np.float32)
        stream_inv = np.zeros(T_total * P, np.float32)
        stream_idx[pos] = (cl_s - j_s * CH).astype(np.int16)
        stream_d[pos] = (rl_s - w_s * P).astype(np.float32)
        stream_inv[pos] = inv_global[rl_s + c * NPC]

        idx_wrapped = np.concatenate(
            [_wrap_idxs(stream_idx[s0 * P:s1 * P]) for (_, _, s0, s1) in segs],
            axis=1)
        d_tiled = stream_d.reshape(T_total, P).T.astype(ml_dtypes.bfloat16)
        inv_tiled = stream_inv.reshape(T_total, P).T.astype(ml_dtypes.bfloat16)
        cores.append((idx_wrapped, d_tiled, inv_tiled))

    return t_wj, tile_off, T_total, segs, cores


def _build_program(t_wj, tile_off, T_total, segs, split_waits=True,
                   do_gather=True, do_compute=True):
    nc = bacc.Bacc("TRN2", num_swdge_queues=4)

    ST16 = sum((s1 - s0) * P // 16 for (_, _, s0, s1) in segs)
    xdup_d = nc.dram_tensor("xdup", [N_NODES, 2 * D], bf16, kind="ExternalInput")
    xt_d = nc.dram_tensor("xt", [D, NPC_PAD], f32r, kind="ExternalInput")
    idx_d = nc.dram_tensor("idx", [P, ST16], i16, kind="ExternalInput")
    d_d = nc.dram_tensor("d", [P, T_total], bf16, kind="ExternalInput")
    inv_d = nc.dram_tensor("inv", [P, T_total], bf16, kind="ExternalInput")
    iota_d = nc.dram_tensor("iota", [P, P], bf16, kind="ExternalInput")
    wa_d = nc.dram_tensor("wa", [D, K * D], f32r, kind="ExternalInput")
    wxi_d = nc.dram_tensor("wxi", [D, K * D], f32r, kind="ExternalInput")
    out_d = nc.dram_tensor("out", [NPC_PAD, K * D], f32, kind="ExternalOutput")

    # gather segment offsets in the idx tensor (int16 columns)
    seg_idx_off = []
    o = 0
    for (_, _, s0, s1) in segs:
        seg_idx_off.append(o)
        o += (s1 - s0) * P // 16

    # batch tile ranges
    batch_range = []
    for b in range(NBATCH):
        bsegs = [s for s in segs if s[0] == b]
        batch_range.append((bsegs[0][2], bsegs[-1][3]))
    S_max = max(s1 - s0 for (s0, s1) in batch_range)

    with tile.TileContext(nc) as tc:
        with (
            tc.tile_pool(name="const", bufs=1) as cpool,
            tc.tile_pool(name="msg", bufs=2) as mpool,
            tc.tile_pool(name="q", bufs=5) as qpool,
            tc.tile_pool(name="agg", bufs=4) as apool,
            tc.tile_pool(name="outs", bufs=4) as opool,
            tc.tile_pool(name="psA", bufs=3, space="PSUM") as pApool,
            tc.tile_pool(name="psB", bufs=3, space="PSUM") as pBpool,
        ):
            idx_t = cpool.tile([P, ST16], i16)
            d_t = cpool.tile([P, T_total], bf16)
            inv_t = cpool.tile([P, T_total], bf16)
            iota_t = cpool.tile([P, P], bf16)
            xt_t = cpool.tile([D, NPC_PAD], f32r)
            wa_t = cpool.tile([D, K * D], f32r)
            wxi_t = cpool.tile([D, K * D], f32r)
            nc.sync.dma_start(out=idx_t[:], in_=idx_d[:])
            nc.sync.dma_start(out=d_t[:], in_=d_d[:])
            nc.sync.dma_start(out=inv_t[:], in_=inv_d[:])
            nc.sync.dma_start(out=iota_t[:], in_=iota_d[:])
            nc.sync.dma_start(out=xt_t[:], in_=xt_d[:])
            nc.sync.dma_start(out=wa_t[:], in_=wa_d[:])
            nc.sync.dma_start(out=wxi_t[:], in_=wxi_d[:])

            gq = 0
            for b in range(NBATCH):
                bs0, bs1 = batch_range[b]
                msg_t = mpool.tile([P, S_max, 2 * D], bf16, tag="msg")

                q_bj = {}
                for si, (bb, j, s0, s1) in enumerate(segs):
                    if bb != b:
                        continue
                    L = (s1 - s0) * P
                    io = seg_idx_off[si]
                    if do_gather:
                        nc.gpsimd.dma_gather(
                            msg_t[:, s0 - bs0:s1 - bs0, :],
                            xdup_d[CH * j:, :] if j else xdup_d[:],
                            idx_t[:, io:io + L // 16],
                            L, L, 2 * D,
                            single_packet=False,
                            queue_num=gq % 4,
                        )
                        gq += 1
                    if not do_compute:
                        continue
                    Sbj = s1 - s0
                    # batched Q build for the whole (batch, chunk) segment:
                    # Q[e, t, i] = (d[e, t] == i), all-bf16
                    q_t = qpool.tile([P, Sbj, P], bf16, tag="q")
                    _ia = iota_t[:]
                    iota_b = bass.AP(_ia.tensor, _ia.offset,
                                     [_ia.ap[0], [0, Sbj], _ia.ap[1]])
                    dcols = d_t[:, s0:s1]
                    d_b = bass.AP(dcols.tensor, dcols.offset,
                                  [dcols.ap[0], dcols.ap[1], [0, P]])
                    nc.vector.tensor_tensor(out=q_t[:], in0=iota_b, in1=d_b,
                                            op=mybir.AluOpType.is_equal)
                    q_bj[j] = (q_t, s0)
                    # batched in-place scale: msg[:, :, 0:64] *= invdeg
                    icols = inv_t[:, s0:s1]
                    i_b = bass.AP(icols.tensor, icols.offset,
                                  [icols.ap[0], icols.ap[1], [0, D]])
                    mseg = msg_t[:, s0 - bs0:s1 - bs0, 0:D]
                    nc.vector.tensor_tensor(out=mseg, in0=mseg, in1=i_b,
                                            op=mybir.AluOpType.mult)

                if not do_gather:
                    nc.vector.memset(msg_t[:, 0, :], 0.0)
                ws = list(range(b * WB, min((b + 1) * WB, NW)))
                if not do_compute:
                    continue
                # window-major consumption: each window's PSUM group is
                # opened and closed consecutively, then fed to stage B.
                for w in ws:
                    psA = pApool.tile([D, P], mybir.dt.float32,
                                      space="PSUM", tag="psA")
                    for j in range(NCH):
                        q_t, qs0 = q_bj[j]
                        for t in range(int(t_wj[w, j])):
                            tt = int(tile_off[w, j]) + t
                            nc.tensor.matmul(
                                out=psA[:],
                                lhsT=msg_t[:, tt - bs0, 0:D],
                                rhs=q_t[:, tt - qs0, :],
                                start=(j == 0 and t == 0),
                                stop=(j == NCH - 1 and t == int(t_wj[w, j]) - 1),
                            )

                    aggT_t = apool.tile([D, P], f32r, tag="agg")
                    nc.scalar.mul(aggT_t[:], psA[:], 1.0)
                    psB = pBpool.tile([P, K * D], mybir.dt.float32,
                                      space="PSUM", tag="psB")
                    nc.tensor.matmul(out=psB[:], lhsT=aggT_t[:], rhs=wa_t[:],
                                     start=True, stop=False)
                    nc.tensor.matmul(out=psB[:],
                                     lhsT=xt_t[:, w * P:(w + 1) * P],
                                     rhs=wxi_t[:], start=False, stop=True)
                    out_t = opool.tile([P, K * D], f32, tag="out")
                    nc.scalar.mul(out_t[:], psB[:], 1.0)
                    nc.sync.dma_start(out=out_d[w * P:(w + 1) * P, :],
                                      in_=out_t[:])

    nc.compile()
    if split_waits:
        _split_sync_waits(nc)
    return nc


def kernel(x, edge_index, weights):
    x = np.asarray(x, dtype=np.float32)
    weights = np.asarray(weights, dtype=np.float32)

    t_wj, tile_off, T_total, segs, cores = _build_plan(edge_index)

    shape_key = (T_total, tuple(t_wj.ravel().tolist()))
    if shape_key in _CACHE:
        nc = _CACHE[shape_key]
    else:
        nc = _build_program(t_wj, tile_off, T_total, segs)
        _CACHE.clear()
        _CACHE[shape_key] = nc

    x_bf = x.astype(ml_dtypes.bfloat16)
    xdup = np.concatenate([x_bf, x_bf], axis=1)          # [N, 128] bf16

    iota = np.broadcast_to(
        np.arange(P, dtype=np.float32), (P, P)).astype(ml_dtypes.bfloat16)

    wa = np.ascontiguousarray(
        weights[:, :D, :].transpose(1, 0, 2).reshape(D, K * D))
    wx = weights[:, D:, :].transpose(1, 0, 2).reshape(D, K * D).copy()
    eye = np.eye(D, dtype=np.float32)
    for k in range(K):
        wx[:, k * D:(k + 1) * D] += eye

    in_maps = []
    for c in range(N_CORES):
        idx_wrapped, d_tiled, inv_tiled = cores[c]
        xt = np.zeros((D, NPC_PAD), np.float32)
        xt[:, :NPC] = x[c * NPC:(c + 1) * NPC].T
        in_maps.append({
            "xdup": xdup,
            "xt": xt,
            "idx": idx_wrapped,
            "d": d_tiled,
            "inv": inv_tiled,
            "iota": np.ascontiguousarray(iota),
            "wa": wa,
            "wxi": wx,
        })

    res = run_bass_kernel_spmd(nc, in_maps, core_ids=list(range(N_CORES)))

    out = np.empty((N_NODES, K, D), np.float32)
    for c in range(N_CORES):
        oc = res.results[c]["out"][:NPC]                 # [12500, 512]
        out[c * NPC:(c + 1) * NPC] = oc.reshape(NPC, K, D)
    return out
